# revision 28
# baseline (speedup 1.0000x reference)
"""Cross-attention kernel for TRN2, 8 NeuronCores.

Sharding: core (b, g) = batch b (4) x head-group g (2 groups of 4 heads).
Each core computes q/k/v projections for its 4 heads on its batch, full
T x (T+2) attention for those heads, and a partial output projection
(contribution of its 4 heads to out = attn @ Wo.T). Host sums the two
partials per batch and adds the constant (bo + Wo @ bv) term.

Math notes (vs reference):
  - 1/sqrt(Dh) folded into Wq/bq host-side.
  - tanh(g) folded into the advisory-token stream host-side
    (hpTs = hp * tanh(g), bkad = bk * tanh(g)).
  - softmax computed without max-subtraction (scores are O(5), exp is
    safe in fp32/bf16 range for this data distribution).
  - v-bias handled exactly on host: since rows of softmax sum to 1,
    its contribution to the output is the constant Wo @ bv.
  - all matmuls in bf16 with fp32 PSUM accumulation.
  - softmax denominator: sum+broadcast fused into one matmul with an
    all-ones [128,128] lhsT; reciprocal on the vector engine.
  - partial outputs returned in bf16; host upcasts and sums.

Schedule (single pass, engines balanced):
  - input DMAs in consumption order on three queues
    (sync HWDGE: wk,wq,biases; scalar HWDGE: xT,wo; gpsimd SWDGE: wv).
  - phase A: k-proj (all t), k_ad, q-proj(t-tile 0), v-proj, v_ad.
  - phase B/C per t-tile: attention for 4 heads (scores pairs ->
    one [128,1024] exp -> AV), q-proj for the next t-tile interleaved,
    then output projection + out-DMA for this t-tile.
"""

import math
import numpy as np
import ml_dtypes

import concourse.bass as bass
import concourse.mybir as mybir
import concourse.tile as tile
from concourse import bacc
from concourse.bass_utils import run_bass_kernel_spmd

BF16 = mybir.dt.bfloat16
F32 = mybir.dt.float32
AFT = mybir.ActivationFunctionType

P = 128
B, T, DIM = 4, 2048, 1024
NH, DH = 8, 128
HPG = 4              # heads per core
GD = HPG * DH        # 512 out-dims per core
KC = DIM // P        # 8 contraction chunks of the model dim
TT = 512             # t tile for attention
NT = T // TT         # 4 t tiles
NTC = T // P         # 16 t chunks of 128 (v layout, o-proj)
SFC = T // P         # 16 full s-chunks (key chunks of 128)
NPR = SFC // 2       # 8 score-chunk pairs

_CACHE = {}


def _build():
    nc = bacc.Bacc(
        "TRN2", target_bir_lowering=False, debug=False, enable_asserts=False
    )

    # Inputs are pre-shuffled host-side so every DMA reads large
    # contiguous per-partition segments (partition-major layouts).
    d = {}
    for name, shape, dt in [
        ("xT", [P, NT, KC, TT], BF16),
        ("wqT", [P, KC, GD], BF16),
        ("wkT", [P, KC, GD], BF16),
        ("wvT", [P, KC, GD], BF16),
        ("woT", [P, HPG, DIM], BF16),
        ("bqv", [P, HPG], F32),
        ("bkv", [P, HPG], F32),
        ("bkad", [P, HPG], F32),
        ("hpT", [P, KC, 2], BF16),
        ("hpTs", [P, KC, 2], BF16),
    ]:
        d[name] = nc.dram_tensor(name, shape, dt, kind="ExternalInput").ap()
    out_ap = nc.dram_tensor("out", [T, DIM], BF16, kind="ExternalOutput").ap()

    with tile.TileContext(nc) as tc:
        with (
            tc.tile_pool(name="big", bufs=1) as big,
            tc.tile_pool(name="expp", bufs=4) as expp,
            tc.tile_pool(name="ettp", bufs=3) as ettp,
            tc.tile_pool(name="accp", bufs=2) as accp,
            tc.tile_pool(name="accm", bufs=2) as accm,
            tc.tile_pool(name="rcp", bufs=2) as rcp,
            tc.tile_pool(name="ostg", bufs=3) as ostg,
        ):
            # ---- persistent SBUF residents ----
            xt = big.tile([P, KC, T], BF16)
            wq = big.tile([P, KC, GD], BF16)
            wk = big.tile([P, KC, GD], BF16)
            wv = big.tile([P, KC, GD], BF16)
            wo = big.tile([P, HPG, DIM], BF16)
            bq_s = big.tile([P, HPG], F32)
            bk_s = big.tile([P, HPG], F32)
            bkad_s = big.tile([P, HPG], F32)
            hpt = big.tile([P, KC, 2], BF16)
            hpts = big.tile([P, KC, 2], BF16)
            qt = big.tile([P, HPG, T], BF16)
            kt = big.tile([P, HPG, T + 2], BF16)
            vsb = big.tile([P, NTC, GD], BF16)
            vad = big.tile([2, GD], BF16)
            ot = big.tile([P, HPG, T], BF16)
            ones128 = big.tile([P, P], BF16)

            # ---- input DMAs ----
            # The SDMA engines round-robin between queues with pending
            # work, so later-needed tensors must queue strictly BEHIND the
            # first k-chain's data (wk + xT tile 0) or they steal HBM
            # bandwidth from it. wk/xt0 are interleaved per-chunk across
            # the two HWDGE rings in chain-consumption order (c0 first on
            # both rings).
            for c in range(KC):
                eng = nc.sync if (c // 2) % 2 == 0 else nc.scalar
                oth = nc.scalar if (c // 2) % 2 == 0 else nc.sync
                eng.dma_start(wk[:, c : c + 1, :], d["wkT"][:, c : c + 1, :])
                oth.dma_start(xt[:, c : c + 1, 0:TT], d["xT"][:, 0, c : c + 1, :])
            # then, per ring, in consumption order
            nc.sync.dma_start(bk_s[:], d["bkv"][:])
            nc.sync.dma_start(bq_s[:], d["bqv"][:])
            nc.sync.dma_start(bkad_s[:], d["bkad"][:])
            nc.sync.dma_start(hpt[:], d["hpT"][:])
            nc.sync.dma_start(hpts[:], d["hpTs"][:])
            nc.sync.dma_start(wq[:], d["wqT"][:])
            nc.sync.dma_start(wv[:], d["wvT"][:])
            for tti in range(1, NT):
                ts = slice(tti * TT, (tti + 1) * TT)
                nc.scalar.dma_start(xt[:, :, ts], d["xT"][:, tti, :, :])
            nc.scalar.dma_start(wo[:], d["woT"][:])
            nc.vector.memset(ones128[:], 1.0)

            # ---- phase A ----
            with tc.tile_pool(name="psP", bufs=4, space="PSUM") as psP:
                # HAM warmup: ~4.5us of dummy matmuls so the PE clock gate
                # opens (1.2 -> 2.4 GHz) before the first real chain.
                wps = psP.tile([P, TT], F32, tag="ppsum", name="warm")
                for i in range(28):
                    nc.tensor.matmul(wps[:, 0:P], ones128[:], ones128[:],
                                     start=True, stop=True)

                def qk_proj(h, tti, w, bias, dst):
                    ts = slice(tti * TT, (tti + 1) * TT)
                    ps = psP.tile([P, TT], F32, tag="ppsum",
                                  name=f"pp_{h}_{tti}")
                    for c in range(KC):
                        nc.tensor.matmul(
                            ps[:],
                            w[:, c, h * P : (h + 1) * P],
                            xt[:, c, ts],
                            start=(c == 0),
                            stop=(c == KC - 1),
                        )
                    nc.vector.tensor_scalar_add(
                        dst[:, h, ts], ps[:], bias[:, h : h + 1]
                    )

                # k for all t (attention needs every key), then k_ad
                for tti in range(NT):
                    for h in range(HPG):
                        qk_proj(h, tti, wk, bk_s, kt)
                for h in range(HPG):
                    ps2 = psP.tile([P, 2], F32, tag="adsum", name=f"kad_{h}")
                    for c in range(KC):
                        nc.tensor.matmul(
                            ps2[:],
                            wk[:, c, h * P : (h + 1) * P],
                            hpts[:, c, :],
                            start=(c == 0),
                            stop=(c == KC - 1),
                        )
                    nc.vector.tensor_scalar_add(
                        kt[:, h, T : T + 2], ps2[:], bkad_s[:, h : h + 1]
                    )
                # q for t-tile 0 only; the rest interleaves with attention
                for h in range(HPG):
                    qk_proj(h, 0, wq, bq_s, qt)
                # v for all t, plus advisory v
                for tci in range(NTC):
                    ps = psP.tile([P, GD], F32, tag="ppsum", name=f"vp_{tci}")
                    for c in range(KC):
                        nc.tensor.matmul(
                            ps[:],
                            xt[:, c, tci * P : (tci + 1) * P],
                            wv[:, c, :],
                            start=(c == 0),
                            stop=(c == KC - 1),
                        )
                    nc.vector.tensor_copy(vsb[:, tci, :], ps[:])
                vps = psP.tile([2, GD], F32, tag="adsum", name="vad_ps")
                for c in range(KC):
                    nc.tensor.matmul(
                        vps[:], hpt[:, c, :], wv[:, c, :],
                        start=(c == 0), stop=(c == KC - 1),
                    )
                nc.vector.tensor_copy(vad[:], vps[:])

            # ---- phase B/C: attention + o-proj per t-tile ----
            # Deferred-work queue: each head's denominator/normalize, the
            # next t-tile's q-projection, and the o-proj chunks are emitted
            # one item at a time between score pairs of LATER heads, so the
            # PE never sits waiting on the scalar/vector dependency chains.
            from collections import deque
            pending = deque()

            def drain(n):
                for _ in range(min(n, len(pending))):
                    pending.popleft()()

            with (
                tc.tile_pool(name="psS", bufs=2, space="PSUM") as psS,
                tc.tile_pool(name="psO", bufs=2, space="PSUM") as psO,
                tc.tile_pool(name="paux", bufs=2, space="PSUM") as paux,
            ):
                def make_qnext(h, tti):
                    def emit():
                        ts = slice(tti * TT, (tti + 1) * TT)
                        ps = paux.tile([P, TT], F32, tag="aux",
                                       name=f"qn_{h}_{tti}")
                        for c in range(KC):
                            nc.tensor.matmul(
                                ps[:],
                                wq[:, c, h * P : (h + 1) * P],
                                xt[:, c, ts],
                                start=(c == 0),
                                stop=(c == KC - 1),
                            )
                        nc.vector.tensor_scalar_add(
                            qt[:, h, ts], ps[:], bq_s[:, h : h + 1]
                        )
                    return emit

                def make_denom(h, tti, ops, accP):
                    def emit():
                        ts = slice(tti * TT, (tti + 1) * TT)
                        accM = accm.tile([P, TT], BF16, tag="summ",
                                         name=f"am_{h}_{tti}")
                        nc.vector.tensor_add(accM[:], accP[:, 0, :],
                                             accP[:, 1, :])
                        dps = paux.tile([P, TT], F32, tag="aux",
                                        name=f"d_{h}_{tti}")
                        nc.tensor.matmul(dps[:], ones128[:], accM[:],
                                         start=True, stop=True)
                        rcb = rcp.tile([P, TT], F32, tag="recip",
                                       name=f"rc_{h}_{tti}")
                        nc.vector.reciprocal_approx_fast(rcb[:], dps[:])
                        nc.vector.tensor_mul(ot[:, h, ts], ops[:], rcb[:])
                    return emit

                def make_oproj(tci):
                    def emit():
                        tcs = slice(tci * P, (tci + 1) * P)
                        stg = ostg.tile([P, DIM], BF16, tag="ostage",
                                        name=f"o_{tci}")
                        for half in range(2):
                            cps = paux.tile([P, 512], F32, tag="aux",
                                            name=f"op_{tci}_{half}")
                            for c in range(HPG):
                                nc.tensor.matmul(
                                    cps[:],
                                    ot[:, c, tcs],
                                    wo[:, c, half * 512 : (half + 1) * 512],
                                    start=(c == 0),
                                    stop=(c == HPG - 1),
                                )
                            nc.vector.tensor_copy(
                                stg[:, half * 512 : (half + 1) * 512], cps[:]
                            )
                        nc.sync.dma_start(out_ap[tcs, :], stg[:])
                    return emit

                for tti in range(NT):
                    ts = slice(tti * TT, (tti + 1) * TT)
                    for h in range(HPG):
                        hs = slice(h * P, (h + 1) * P)
                        # advisory-token tail scores first
                        tps = paux.tile([2, TT], F32, tag="aux",
                                        name=f"st_{h}_{tti}")
                        nc.tensor.matmul(tps[:], kt[:, h, T : T + 2],
                                         qt[:, h, ts], start=True, stop=True)
                        ett = ettp.tile([2, TT], BF16, tag="exptail",
                                        name=f"et_{h}_{tti}")
                        nc.scalar.activation(ett[:], tps[:], AFT.Exp)

                        ops = psO.tile([P, TT], F32, tag="avacc",
                                       name=f"av_{h}_{tti}")
                        accP = accp.tile([P, 2, TT], BF16, tag="sumacc",
                                         name=f"acc_{h}_{tti}")
                        # AV matmuls run one pair behind the scores/exp so
                        # the PE never waits on the exp of the current pair.
                        etps = []
                        for j in range(NPR):
                            sps = psS.tile([P, 2, TT], F32, tag="scores",
                                           name=f"s_{h}_{tti}_{j}")
                            for k in range(2):
                                sc = 2 * j + k
                                nc.tensor.matmul(
                                    sps[:, k, :],
                                    kt[:, h, sc * P : (sc + 1) * P],
                                    qt[:, h, ts], start=True, stop=True,
                                )
                            etp = expp.tile([P, 2, TT], BF16, tag="exp",
                                            name=f"e_{h}_{tti}_{j}")
                            nc.scalar.activation(etp[:], sps[:], AFT.Exp)
                            if j == 0:
                                nc.vector.tensor_copy(accP[:], etp[:])
                            else:
                                nc.vector.tensor_add(accP[:], accP[:], etp[:])
                            etps.append(etp)
                            if j >= 1:
                                for k in range(2):
                                    sc = 2 * (j - 1) + k
                                    nc.tensor.matmul(
                                        ops[:], vsb[:, sc, hs],
                                        etps[j - 1][:, k, :],
                                        start=(sc == 0), stop=False,
                                    )
                            if j in (1, 3, 5):
                                drain(1)
                        for k in range(2):
                            sc = 2 * (NPR - 1) + k
                            nc.tensor.matmul(
                                ops[:], vsb[:, sc, hs],
                                etps[NPR - 1][:, k, :],
                                start=False, stop=False,
                            )
                        # advisory tail: exp into acc rows 0:2, AV finish
                        nc.vector.tensor_add(accP[0:2, 0, :], accP[0:2, 0, :],
                                             ett[:])
                        nc.tensor.matmul(ops[:], vad[:, hs], ett[:],
                                         start=False, stop=True)
                        pending.append(make_denom(h, tti, ops, accP))
                        if tti + 1 < NT:
                            pending.append(make_qnext(h, tti + 1))
                    for t4 in range(4):
                        pending.append(make_oproj(tti * 4 + t4))
                drain(len(pending))

    nc.compile()
    return nc


def _get_nc():
    if "nc" not in _CACHE:
        _CACHE["nc"] = _build()
    return _CACHE["nc"]


def kernel(x, h, p, Wq, bq, Wk, bk, Wv, bv, Wo, bo, g, **_):
    x = np.asarray(x, np.float32)
    h = np.asarray(h, np.float32)
    p = np.asarray(p, np.float32)
    Wq = np.asarray(Wq, np.float32)
    bq = np.asarray(bq, np.float32)
    Wk = np.asarray(Wk, np.float32)
    bk = np.asarray(bk, np.float32)
    Wv = np.asarray(Wv, np.float32)
    bv = np.asarray(bv, np.float32)
    Wo = np.asarray(Wo, np.float32)
    bo = np.asarray(bo, np.float32)
    g = np.asarray(g, np.float32)

    nc = _get_nc()
    bf = ml_dtypes.bfloat16
    s = 1.0 / math.sqrt(DH)
    gt = float(np.tanh(g[0]))
    hp = np.concatenate([h, p], axis=1)  # [B, 2, DIM]

    def shuf_w(wT):
        # [DIM, F] -> [P, KC, F] : partition-major, contiguous per partition
        return np.ascontiguousarray(
            wT.reshape(KC, P, -1).transpose(1, 0, 2)).astype(bf)

    per_group = []
    for gi in range(2):
        sl = slice(gi * GD, (gi + 1) * GD)
        per_group.append({
            "wqT": shuf_w((Wq[sl] * s).T),
            "wkT": shuf_w(Wk[sl].T),
            "wvT": shuf_w(Wv[sl].T),
            "woT": np.ascontiguousarray(
                Wo[:, sl].T.reshape(HPG, P, DIM).transpose(1, 0, 2)
            ).astype(bf),
            "bqv": np.ascontiguousarray((bq[sl] * s).reshape(HPG, P).T,
                                        dtype=np.float32),
            "bkv": np.ascontiguousarray(bk[sl].reshape(HPG, P).T,
                                        dtype=np.float32),
            "bkad": np.ascontiguousarray((bk[sl] * gt).reshape(HPG, P).T,
                                         dtype=np.float32),
        })

    in_maps = []
    for b in range(B):
        # x[b].T is [DIM, T] -> [P, NT, KC, TT]
        xTb = np.ascontiguousarray(
            x[b].T.reshape(KC, P, NT, TT).transpose(1, 2, 0, 3)).astype(bf)
        hpTb = np.ascontiguousarray(
            hp[b].T.reshape(KC, P, 2).transpose(1, 0, 2)).astype(bf)
        hpTsb = np.ascontiguousarray(
            (hp[b] * gt).T.reshape(KC, P, 2).transpose(1, 0, 2)).astype(bf)
        for gi in range(2):
            m = dict(per_group[gi])
            m["xT"] = xTb
            m["hpT"] = hpTb
            m["hpTs"] = hpTsb
            in_maps.append(m)

    _CACHE["last_in_maps"] = in_maps
    res = run_bass_kernel_spmd(nc, in_maps, list(range(8)))
    outs = res.results

    const = (bo + Wo @ bv).astype(np.float32)
    out = np.empty((B, T, DIM), np.float32)
    for b in range(B):
        out[b] = (outs[2 * b]["out"].astype(np.float32)
                  + outs[2 * b + 1]["out"].astype(np.float32) + const)
    return out
